# revision 3
# baseline (speedup 1.0000x reference)
"""BitMGQA forward on 8 trn2 NeuronCores — zero-communication version.

Core c owns batch b=c//4 and query rows (c%4)*512:(c%4+1)*512. Every core
recomputes the FULL K/V projections for its batch (T=2048 keys) instead of
all-gathering the per-core slices: +0.07ms of extra matmul per core buys
removal of all three collectives (each a cross-core sync point that stalls
the NEFF far longer than the recomputed FLOPs).

All matmul operands are bf16 (FWL-eligible stationaries); PSUM accumulation,
softmax statistics, and LayerNorm statistics stay fp32/f32r. Softmax
denominators come from a DVE-side running sum (1 matmul per head instead of
16). Outputs are disjoint row slices -> host concat, no cross-core traffic.
"""

import contextlib

import numpy as np

import concourse.bacc as bacc
import concourse.mybir as mybir
import concourse.tile as tile
from concourse.bass_utils import run_bass_kernel_spmd

B, T, C = 2, 2048, 2048
H, KV = 16, 4
HD = C // H  # 128
KVC = HD * KV  # 512
EPS = 1e-5
R = 512  # query rows per core
N_CORES = 8
SCALE = 1.0 / np.sqrt(HD)

F32 = mybir.dt.float32
F32R = mybir.dt.float32r
BF16 = mybir.dt.bfloat16
AF = mybir.ActivationFunctionType
ALU = mybir.AluOpType


def build_kernel(loop_n=1):
    nc = bacc.Bacc(
        "TRN2", target_bir_lowering=False, debug=False, num_devices=N_CORES
    )

    # Per-core inputs (host pre-transposed/tiled, see kernel() below)
    xq_d = nc.dram_tensor("xq", [128, 16, R], BF16, kind="ExternalInput").ap()
    xk_d = nc.dram_tensor("xk", [4, 128, 16, 512], BF16, kind="ExternalInput").ap()
    xv_d = nc.dram_tensor("xv", [4, 128, 16, 512], BF16, kind="ExternalInput").ap()
    wq_d = nc.dram_tensor("wq", [16, 128, 16, 128], BF16, kind="ExternalInput").ap()
    wk_d = nc.dram_tensor("wk", [128, 4, 16, 128], BF16, kind="ExternalInput").ap()
    wv_d = nc.dram_tensor("wv", [128, 16, KVC], BF16, kind="ExternalInput").ap()
    wo_d = nc.dram_tensor("wo", [4, 128, 16, 512], BF16, kind="ExternalInput").ap()
    bq_d = nc.dram_tensor("bq", [128, 16], F32, kind="ExternalInput").ap()
    bk_d = nc.dram_tensor("bk", [128, 4], F32, kind="ExternalInput").ap()
    bv_d = nc.dram_tensor("bv", [128, 4], F32, kind="ExternalInput").ap()
    lnw_d = nc.dram_tensor("lnw", [128, 16], F32, kind="ExternalInput").ap()
    lnb_d = nc.dram_tensor("lnb", [128, 16], F32, kind="ExternalInput").ap()
    ones_d = nc.dram_tensor("ones", [128, 1], BF16, kind="ExternalInput").ap()
    oc_d = nc.dram_tensor("oc", [128, 1], F32R, kind="ExternalInput").ap()
    onesr_d = nc.dram_tensor("onesr", [1, 128], F32R, kind="ExternalInput").ap()

    out_d = nc.dram_tensor("out", [R, C], F32, kind="ExternalOutput").ap()

    with tile.TileContext(nc) as tc:
        with (
            tc.tile_pool(name="consts", bufs=1) as consts,
            tc.tile_pool(name="xbig", bufs=3) as xbig,        # [128,16,512] bf16
            tc.tile_pool(name="wbig", bufs=3) as wbig,        # [128,16,512] bf16
            tc.tile_pool(name="wqp", bufs=2) as wqp,          # [128,16,128] bf16
            tc.tile_pool(name="qtb", bufs=16) as qtb_pool,    # [128,512] bf16
            tc.tile_pool(name="ktf", bufs=4) as ktf_pool,     # [128,2048] bf16
            tc.tile_pool(name="vfb", bufs=16) as vfb_pool,    # [128,512] bf16
            tc.tile_pool(name="blk", bufs=6) as blk,         # [128,512] bf16
            tc.tile_pool(name="blkf", bufs=5) as blkf,        # [128,512] f32
            tc.tile_pool(name="ytp", bufs=1) as ytp,          # [128,16,512] f32r
            tc.tile_pool(name="s1", bufs=3) as s1,            # [1,512] f32
            tc.tile_pool(name="ps", bufs=4, space="PSUM") as ps,    # [128,512]
            tc.tile_pool(name="psy", bufs=2, space="PSUM") as psy,  # [128,512]
            tc.tile_pool(name="pss", bufs=2, space="PSUM") as pss,  # [1,512]
        ):
            for _it in range(loop_n):
                # ---- first inputs: K chunk 0 + Wk (gate the K-proj start) ----
                xk0 = xbig.tile([128, 16, 512], BF16, tag="xbig", name="xk0")
                nc.sync.dma_start(out=xk0[:], in_=xk_d[0])
                wk = wbig.tile([128, 4, 16, 128], BF16, tag="wbig", name="wk")
                nc.sync.dma_start(out=wk[:], in_=wk_d[:])
                wv = wbig.tile([128, 16, KVC], BF16, tag="wbig", name="wv")
                nc.sync.dma_start(out=wv[:], in_=wv_d[:])

                # ---- constants ----
                ones_col = consts.tile([128, 1], BF16)
                nc.sync.dma_start(out=ones_col[:], in_=ones_d[:])
                oc_col = consts.tile([128, 1], F32R)
                nc.sync.dma_start(out=oc_col[:], in_=oc_d[:])
                ones_row = consts.tile([1, 128], F32R)
                nc.sync.dma_start(out=ones_row[:], in_=onesr_d[:])
                bq_sb = consts.tile([128, 16], F32)
                nc.sync.dma_start(out=bq_sb[:], in_=bq_d[:])
                bk_sb = consts.tile([128, 4], F32)
                nc.sync.dma_start(out=bk_sb[:], in_=bk_d[:])
                bv_sb = consts.tile([128, 4], F32)
                nc.sync.dma_start(out=bv_sb[:], in_=bv_d[:])
                lnw_sb = consts.tile([128, 16], F32)
                nc.sync.dma_start(out=lnw_sb[:], in_=lnw_d[:])
                lnb_sb = consts.tile([128, 16], F32)
                nc.sync.dma_start(out=lnb_sb[:], in_=lnb_d[:])

                # ---- K^T projection, full T ----
                # ktfg[g] = [128 (hd ch of kv-head g), 2048 (keys)] bf16
                ktfg = [
                    ktf_pool.tile([128, T], BF16, tag="ktfg", name=f"ktfg{g}")
                    for g in range(4)
                ]
                xk_t = xk0
                for rt in range(4):
                    if rt > 0:
                        xk_t = xbig.tile([128, 16, 512], BF16, tag="xbig",
                                         name=f"xk{rt}")
                        nc.sync.dma_start(out=xk_t[:], in_=xk_d[rt])
                    for g in range(4):
                        ps_k = ps.tile([128, 512], F32, tag="ps")
                        for i in range(16):
                            nc.tensor.matmul(
                                ps_k[:], wk[:, g, i, :], xk_t[:, i, :],
                                start=(i == 0), stop=(i == 15),
                                skip_group_check=True,
                            )
                        nc.scalar.activation(
                            ktfg[g][:, rt * 512:(rt + 1) * 512], ps_k[:],
                            AF.Identity, bias=bk_sb[:, g:g + 1],
                        )

                # ---- V projection, full T ----
                # vfr[kt] = [128 (key rows), 512 (v ch)] bf16
                vfr = []
                for rt in range(4):
                    xv_t = xbig.tile([128, 16, 512], BF16, tag="xbig",
                                     name=f"xv{rt}")
                    nc.sync.dma_start(out=xv_t[:], in_=xv_d[rt])
                    for k4 in range(4):
                        ps_v = ps.tile([128, 512], F32, tag="ps")
                        for i in range(16):
                            nc.tensor.matmul(
                                ps_v[:], xv_t[:, i, k4 * 128:(k4 + 1) * 128],
                                wv[:, i, :], start=(i == 0), stop=(i == 15),
                                skip_group_check=True,
                            )
                        vt = vfb_pool.tile([128, KVC], BF16, tag="vfr",
                                           name=f"vfr{rt*4+k4}")
                        nc.scalar.activation(vt[:], ps_v[:], AF.Copy)
                        vfr.append(vt)

                # ---- Q^T projection (scale folded into Wq by host) ----
                xq = xbig.tile([128, 16, R], BF16, tag="xbig", name="xq")
                nc.sync.dma_start(out=xq[:], in_=xq_d[:])
                qtbh = []
                for j in range(16):
                    wqb = wqp.tile([128, 16, 128], BF16, tag="wq")
                    nc.sync.dma_start(out=wqb[:], in_=wq_d[j])
                    ps_q = ps.tile([128, 512], F32, tag="ps")
                    for i in range(16):
                        nc.tensor.matmul(
                            ps_q[:], wqb[:, i, :], xq[:, i, :],
                            start=(i == 0), stop=(i == 15),
                            skip_group_check=True,
                        )
                    qh = qtb_pool.tile([128, R], BF16, tag="qtbh",
                                       name=f"qtbh{j}")
                    nc.scalar.activation(
                        qh[:], ps_q[:], AF.Identity, bias=bq_sb[:, j:j + 1]
                    )
                    qtbh.append(qh)

                # ---- prefetch Wo blocks into freed wbig slots ----
                wob = [None] * 4
                for jb in range(3):
                    wob[jb] = wbig.tile([128, 16, 512], BF16, tag="wbig",
                                        name=f"wob{jb}")
                    nc.sync.dma_start(out=wob[jb][:], in_=wo_d[jb])

                # ---- attention (softmax denom via DVE running sum) ----
                yt = ytp.tile([128, 16, R], F32R)
                ps_mu = pss.tile([1, 512], F32, tag="pss")
                ps_sq = pss.tile([1, 512], F32, tag="pss")
                accs = [None] * (2 * H)
                ps_ys = [None] * H

                def fin_stats(h):
                    # softmax-normalize head h and fold it into the LN sums.
                    # Called during head h+1's score phase so the in-order PE
                    # never waits on the DVE denominator chain.
                    g = h // 4
                    rS = s1.tile([1, 512], F32R, tag="s1", name=f"rS{h}")
                    with nc.allow_low_precision("fp32r for bcast matmul"):
                        nc.vector.reciprocal(rS[:], accs[h + H][:])
                    ps_r = ps.tile([128, 512], F32, tag="ps", name=f"ps_r{h}")
                    nc.tensor.matmul(
                        ps_r[:], ones_row[0:1, 0:128], rS[:],
                        start=True, stop=True,
                    )
                    rSb = blkf.tile([128, 512], F32, tag="blkf",
                                    name=f"rSb{h}")
                    nc.vector.tensor_copy(rSb[:], ps_r[:])
                    nc.vector.tensor_tensor(
                        yt[:, h, :], ps_ys[h][:], rSb[:], op=ALU.mult
                    )
                    nc.vector.tensor_scalar_add(
                        yt[:, h, :], yt[:, h, :], bv_sb[:, g:g + 1]
                    )
                    ysq = blkf.tile([128, 512], F32R, tag="blkf",
                                    name=f"ysq{h}")
                    nc.vector.tensor_tensor(
                        ysq[:], yt[:, h, :], yt[:, h, :], op=ALU.mult
                    )
                    nc.tensor.matmul(
                        ps_mu[:], oc_col[:], yt[:, h, :],
                        start=(h == 0), stop=(h == 15), skip_group_check=True,
                    )
                    nc.tensor.matmul(
                        ps_sq[:], oc_col[:], ysq[:],
                        start=(h == 0), stop=(h == 15), skip_group_check=True,
                    )

                for h in range(H):
                    g = h // 4
                    ps_y = psy.tile([128, 512], F32, tag="psy")
                    ps_ys[h] = ps_y
                    acc = blk.tile([128, 512], BF16, tag="blk",
                                   name=f"acc{h}")
                    accs[h] = acc
                    att = [None] * 16

                    def score(kt):
                        ps_s = ps.tile([128, 512], F32, tag="ps")
                        nc.tensor.matmul(
                            ps_s[:], ktfg[g][:, kt * 128:(kt + 1) * 128],
                            qtbh[h][:], start=True, stop=True,
                        )
                        a = blk.tile([128, 512], BF16, tag="blk")
                        nc.scalar.activation(a[:], ps_s[:], AF.Exp)
                        att[kt] = a
                        if kt == 0:
                            nc.vector.tensor_copy(acc[:], a[:])
                        else:
                            nc.vector.tensor_tensor(
                                acc[:], acc[:], a[:], op=ALU.add
                            )

                    def ymm(kt):
                        nc.tensor.matmul(
                            ps_y[:], vfr[kt][:, g * 128:(g + 1) * 128],
                            att[kt][:], start=(kt == 0), stop=(kt == 15),
                            skip_group_check=True,
                        )

                    score(0)
                    score(1)
                    if h > 0:
                        # denominator of the PREVIOUS head: PE does this while
                        # ACT computes this head's first exps
                        ps_S = ps.tile([1, 512], F32, tag="ps",
                                       name=f"ps_S{h-1}")
                        nc.tensor.matmul(
                            ps_S[:], ones_col[:], accs[h - 1][:],
                            start=True, stop=True,
                        )
                        accs[h - 1 + H] = ps_S
                    score(2)
                    ymm(0)
                    score(3)
                    ymm(1)
                    if h > 0:
                        fin_stats(h - 1)
                    for kt in range(4, 16):
                        score(kt)
                        ymm(kt - 2)
                    ymm(14)
                    ymm(15)

                # final head's denominator + stats
                ps_S = ps.tile([1, 512], F32, tag="ps", name="ps_S15")
                nc.tensor.matmul(
                    ps_S[:], ones_col[:], accs[15][:], start=True, stop=True,
                )
                accs[15 + H] = ps_S
                fin_stats(15)

                # ---- LayerNorm stats + apply -> ytn (bf16) ----
                # stat MMs used 1/C weights, so ps_mu = E[y], ps_sq = E[y^2]
                mu = s1.tile([1, 512], F32R, tag="s1", name="mu")
                nc.vector.tensor_copy(mu[:], ps_mu[:])
                mu2 = s1.tile([1, 512], F32, tag="s1", name="mu2")
                nc.vector.tensor_tensor(mu2[:], mu[:], mu[:], op=ALU.mult)
                var = s1.tile([1, 512], F32, tag="s1", name="var")
                nc.vector.tensor_tensor(var[:], ps_sq[:], mu2[:],
                                        op=ALU.subtract)
                nc.vector.tensor_scalar_add(var[:], var[:], EPS)
                sd = s1.tile([1, 512], F32, tag="s1", name="sd")
                nc.scalar.activation(sd[:], var[:], AF.Sqrt)
                rstd = s1.tile([1, 512], F32R, tag="s1", name="rstd")
                with nc.allow_low_precision("fp32r for bcast matmul"):
                    nc.vector.reciprocal(rstd[:], sd[:])
                ps_mb = ps.tile([128, 512], F32, tag="ps")
                nc.tensor.matmul(
                    ps_mb[:], ones_row[0:1, 0:128], mu[:], start=True, stop=True
                )
                mub = blkf.tile([128, 512], F32, tag="blkf", name="mub")
                nc.vector.tensor_copy(mub[:], ps_mb[:])
                # 1/std broadcast stays in PSUM; DVE reads it directly
                rstdb_ps = psy.tile([128, 512], F32, tag="psy")
                nc.tensor.matmul(
                    rstdb_ps[:], ones_row[0:1, 0:128], rstd[:],
                    start=True, stop=True,
                )
                ytn = xbig.tile([128, 16, R], BF16, tag="xbig", name="ytn")

                def ln_apply(ct):
                    # (yt-mu) on GpSimd, *(1/std) and *w+b fused on DVE
                    ytm = blkf.tile([128, 512], F32, tag="blkf",
                                    name=f"ytm{ct}")
                    nc.gpsimd.tensor_sub(ytm[:], yt[:, ct, :], mub[:])
                    scr = blkf.tile([128, 512], F32, tag="blkf",
                                    name=f"scr{ct}")
                    nc.vector.tensor_tensor(
                        scr[:], ytm[:], rstdb_ps[:], op=ALU.mult
                    )
                    nc.vector.tensor_scalar(
                        ytn[:, ct, :], scr[:],
                        lnw_sb[:, ct:ct + 1], lnb_sb[:, ct:ct + 1],
                        op0=ALU.mult, op1=ALU.add,
                    )

                def stage_out(ps_o, jb, m):
                    osb = blkf.tile([128, 512], F32, tag="blkf",
                                    name=f"osb{jb}_{m}")
                    nc.scalar.activation(osb[:], ps_o[:], AF.Copy)
                    nc.sync.dma_start(
                        out=out_d[m * 128:(m + 1) * 128,
                                  jb * 512:(jb + 1) * 512],
                        in_=osb[:],
                    )

                # ---- output projection ----
                # jb=0 runs i-major across 4 PSUM banks, interleaved with the
                # LN apply so each i-step only needs ytn[:, i, :] ready.
                ps_o0 = [
                    ps.tile([128, 512], F32, tag="ps", name=f"ps_o0_{m}")
                    for m in range(4)
                ]
                for i in range(16):
                    ln_apply(i)
                    for m in range(4):
                        nc.tensor.matmul(
                            ps_o0[m][:], ytn[:, i, m * 128:(m + 1) * 128],
                            wob[0][:, i, :], start=(i == 0), stop=(i == 15),
                            skip_group_check=True,
                        )
                for m in range(4):
                    stage_out(ps_o0[m], 0, m)
                for jb in range(1, 4):
                    if wob[jb] is None:
                        wob[jb] = wbig.tile([128, 16, 512], BF16, tag="wbig",
                                            name=f"wob{jb}")
                        nc.sync.dma_start(out=wob[jb][:], in_=wo_d[jb])
                    for m in range(4):
                        ps_o = ps.tile([128, 512], F32, tag="ps")
                        for i in range(16):
                            nc.tensor.matmul(
                                ps_o[:], ytn[:, i, m * 128:(m + 1) * 128],
                                wob[jb][:, i, :], start=(i == 0), stop=(i == 15),
                                skip_group_check=True,
                            )
                        stage_out(ps_o, jb, m)

    nc.compile()
    return nc


_NC_CACHE = None


def _get_nc():
    global _NC_CACHE
    if _NC_CACHE is None:
        _NC_CACHE = build_kernel()
    return _NC_CACHE


def _prep_shared(Wq, bq, Wk, bk, Wv, bv, ln_w, ln_b, Wo, bo):
    import ml_dtypes

    bf = ml_dtypes.bfloat16
    s = np.float32(SCALE)
    WqT = np.ascontiguousarray(Wq.T) * s  # [c, ch], scale folded into q
    wq = np.ascontiguousarray(
        WqT.reshape(16, 128, 16, 128).transpose(2, 1, 0, 3)
    ).astype(bf)
    WkT = np.ascontiguousarray(Wk.T)  # [2048, 512]
    # wk[p, g, i, cc] = WkT[i*128+p, g*128+cc]
    wk = np.ascontiguousarray(
        WkT.reshape(16, 128, 4, 128).transpose(1, 2, 0, 3)
    ).astype(bf)
    WvT = np.ascontiguousarray(Wv.T)  # [2048, 512]
    wv = np.ascontiguousarray(
        WvT.reshape(16, 128, KVC).transpose(1, 0, 2)
    ).astype(bf)
    WoT = np.ascontiguousarray(Wo.T)  # [2048, 2048]
    wo = np.ascontiguousarray(
        WoT.reshape(16, 128, 4, 512).transpose(2, 1, 0, 3)
    ).astype(bf)
    return {
        "wq": wq,
        "wk": wk,
        "wv": wv,
        "wo": wo,
        "bq": np.ascontiguousarray((bq * s).reshape(16, 128).T),
        "bk": np.ascontiguousarray(bk.reshape(4, 128).T),
        "bv": np.ascontiguousarray(bv.reshape(4, 128).T),
        "lnw": np.ascontiguousarray(ln_w.reshape(16, 128).T),
        "lnb": np.ascontiguousarray(ln_b.reshape(16, 128).T),
        "ones": np.ones((128, 1), bf),
        "oc": np.full((128, 1), 1.0 / C, np.float32),
        "onesr": np.ones((1, 128), np.float32),
    }


def _xt_tiled_bf(x, n):
    """x [N_rows, C] -> x^T tiled [n, 128, 16, 512] bf16 (row chunks of 512)."""
    import ml_dtypes

    xT = np.ascontiguousarray(x.T)  # [C, N]
    # [16, 128, n, 512] -> [n, 128, 16, 512]
    t = xT.reshape(16, 128, n, 512).transpose(2, 1, 0, 3)
    return np.ascontiguousarray(t).astype(ml_dtypes.bfloat16)


def _make_in_maps(
    query, key, value, Wq, bq, Wk, bk, Wv, bv, ln_w, ln_b, Wo, bo
):
    query = np.asarray(query, np.float32)
    key = np.asarray(key, np.float32)
    value = np.asarray(value, np.float32)
    shared = _prep_shared(
        np.asarray(Wq, np.float32), np.asarray(bq, np.float32),
        np.asarray(Wk, np.float32), np.asarray(bk, np.float32),
        np.asarray(Wv, np.float32), np.asarray(bv, np.float32),
        np.asarray(ln_w, np.float32), np.asarray(ln_b, np.float32),
        np.asarray(Wo, np.float32), np.asarray(bo, np.float32),
    )
    xk_b = [_xt_tiled_bf(key[b], 4) for b in range(B)]
    xv_b = [_xt_tiled_bf(value[b], 4) for b in range(B)]
    in_maps = []
    for c in range(N_CORES):
        b = c // 4
        r0 = (c % 4) * R
        m = dict(shared)
        m["xq"] = _xt_tiled_bf(query[b, r0:r0 + R, :], 1)[0]
        m["xk"] = xk_b[b]
        m["xv"] = xv_b[b]
        in_maps.append(m)
    return in_maps


def kernel(
    query, key, value, Wq, bq, Wk, bk, Wv, bv, ln_w, ln_b, Wo, bo
):
    import ml_dtypes

    query = np.asarray(query, np.float32)
    key = np.asarray(key, np.float32)
    value = np.asarray(value, np.float32)

    nc = _get_nc()
    in_maps = _make_in_maps(
        query, key, value, Wq, bq, Wk, bk, Wv, bv, ln_w, ln_b, Wo, bo
    )

    import os
    trace_env = os.environ.get("KBENCH_TRACE", "")
    if trace_env:
        res = run_bass_kernel_spmd(
            nc, in_maps, core_ids=list(range(N_CORES)),
            trace=True,
            trace_cores=[int(x) for x in trace_env.split(",")]
            if trace_env != "1" else [0],
        )
        global LAST_RESULTS
        LAST_RESULTS = res
    else:
        res = run_bass_kernel_spmd(nc, in_maps, core_ids=list(range(N_CORES)))

    out = np.empty((B, T, C), np.float32)
    for c in range(N_CORES):
        b = c // 4
        r0 = (c % 4) * R
        out[b, r0:r0 + R, :] = res.results[c]["out"]
    bo_f = np.asarray(bo, np.float32)
    if np.any(bo_f):
        out += bo_f
    return out
